# revision 59
# baseline (speedup 1.0000x reference)
"""Trainium2 Bass kernel for CustomStaticEdgeConv (GNN message passing).

out[n] = mean_{e: row[e]=n} relu( concat(x[n], x[col_e]-x[n]) @ W.T + b )

Math restructure:
    z_e = P[row_e] + Q[col_e],  P = x@(W1-W2).T + b,  Q = x@W2.T
    relu(z_e) = P + max(Q_e, -P)
    out[n] = P[n]*(1 + pad_n/deg_n) + (1/deg_n) * sum_slots max(Q_slot, -P[n])
(padding slots gather a dummy table row whose MLP output is -1e30, so they
contribute -P[n]; the host folds that into the P coefficient and applies the
1/deg scale itself.)

Device pipeline per core (edges sharded by destination node, 6250 nodes/core):
    dma_gather(transpose=True)  -> x[col] feature-major bf16     [GPSIMD/DMA]
      (striped across 4 SWDGE queues; queue-pairs of Q7 cores overlap ~3x)
    matmul(Baug stationary)     -> Q_T in PSUM fp32              [PE]
    activation copy             -> Q_T bf16 in SBUF              [ACT]
    broadcast DMA               -> -P expanded per slot (dense)  [SYNC/DMA]
    tensor_tensor(max, dense)   -> M = max(Q, -P) bf16 @2x mode  [DVE]
    tensor_reduce(add, 3D AP)   -> R_T per virtual node          [DVE]
    dma R_T (feature-major)     -> DRAM                          [SYNC/DMA]
Virtual nodes: each node splits by col-half (int16 gather index limit) and is
grouped with equal-degree peers into 128-wide batches so the segmented reduce
is a constant-stride access pattern. -P is drained from PSUM by ACT with
scale=-1 into bf16 and expanded per-batch by a stride-0 broadcast DMA so the
DVE max sees two dense bf16 SBUF operands (2x perf mode instead of 1x).
"""

import sys

sys.path.insert(0, "/opt/trn_rl_repo")

import numpy as np
import ml_dtypes

import concourse.bass as bass
import concourse.bacc as bacc
import concourse.mybir as mybir
from concourse.bass_utils import run_bass_kernel_spmd
from concourse.library_config import mlp as mlp_lib

# ---------------------------------------------------------------- constants
N_NODES = 50000
F_IN = 64
F_OUT = 128
N_EDGES = 800000
NCORES = 8
LPC = N_NODES // NCORES  # 6250 nodes per core
CLASS_SPLIT = 32000      # col < 32000 -> lo table, else hi table
# x_pad table layout: [dummy_lo, x[0:32000], dummy_hi, x[32000:50000]]
HI_BASE = CLASS_SPLIT + 1                     # row index of dummy_hi
TAB_ROWS = 2 + N_NODES                        # 50002
DUMMY_CH = F_IN                               # one-hot channel of dummy rows
NEG_BIG = -1.0e30

SEG_SLOTS = 6144         # max slots per dma_gather segment (2 fit per ring)
SUB_SLOTS = 1024         # max slots per PSUM subtile
NQ = 4                   # SWDGE queues (Q7 core pairs)

F32 = mybir.dt.float32
BF16 = mybir.dt.bfloat16
I16 = mybir.dt.int16


# ---------------------------------------------------------------- host prep
def _plan_and_pack(edge_index):
    """Build the shared SPMD batch plan and per-core index blobs.

    Returns (plan, per_core) where plan is identical across cores
    (drives codegen) and per_core holds DRAM inputs + assembly metadata.
    """
    rows = np.asarray(edge_index[0], dtype=np.int64)
    cols = np.asarray(edge_index[1], dtype=np.int64)
    core = rows // LPC
    loc_row = (rows - core * LPC).astype(np.int32)
    cls = (cols >= CLASS_SPLIT).astype(np.int32)
    # gather index within class table (dummy row of each class is index 0)
    gidx = np.where(cls == 0, cols + 1, cols - CLASS_SPLIT + 1).astype(np.int32)

    # order edges by (core, class, local_row) -> virtual nodes are runs
    order = np.lexsort((loc_row, cls, core))
    core_s, cls_s, lr_s, gi_s = core[order], cls[order], loc_row[order], gidx[order]

    cores = []
    for c in range(NCORES):
        sel = core_s == c
        cc, ll, gg = cls_s[sel], lr_s[sel], gi_s[sel]
        # virtual node = unique (class, local_row) run
        key = cc.astype(np.int64) * LPC + ll
        ukey, start, vdeg = np.unique(key, return_index=True, return_counts=True)
        vcls = (ukey // LPC).astype(np.int32)
        vnode = (ukey % LPC).astype(np.int32)
        # true degree per local node
        deg = np.bincount(ll, minlength=LPC).astype(np.int64)
        cores.append(dict(cc=cc, ll=ll, gg=gg, start=start, vdeg=vdeg.astype(np.int64),
                          vcls=vcls, vnode=vnode, deg=deg))

    # --- shared batch plan: per class, batches of 128 virtuals sorted by deg
    # desc. Each entry carries its chunk index j into the degree-sorted vnode
    # list so batches can be reordered freely; the smallest class-0 batch is
    # moved to the front so the lead gather segment (and thus the first
    # transfer + downstream pipeline ramp) is small.
    plan_batches = []  # list of (cls, g, chunk_j)
    for h in (0, 1):
        per_core_sorted = []
        for c in range(NCORES):
            d = cores[c]
            m = d["vcls"] == h
            sd = np.sort(d["vdeg"][m])[::-1]
            per_core_sorted.append(sd)
        nb = max((len(s) + 127) // 128 for s in per_core_sorted)
        cls_list = []
        for j in range(nb):
            g = 1
            for s in per_core_sorted:
                if len(s) > j * 128:
                    g = max(g, int(s[j * 128]))
            cls_list.append((h, g, j))
        if h == 0 and len(cls_list) > 1:
            cls_list = [cls_list[-1]] + cls_list[:-1]
        plan_batches.extend(cls_list)

    nbatch = len(plan_batches)
    tot_slots = sum(128 * g for (_h, g, _j) in plan_batches)
    assert tot_slots % 16 == 0

    # --- segments: runs of same-class batches, <= SEG_SLOTS slots each,
    # sized near-equal within each class so the 4 SWDGE queue pairs get
    # balanced desc-gen work (~7.7ns/idx/pair is the pipeline pacer). The
    # first segment is ~half size so the first trigger fires early.
    segments = []  # (cls, slot_start, nslots)
    off = 0
    for h in (0, 1):
        cls_sizes = [128 * g for (hh, g, _j) in plan_batches if hh == h]
        slots_c = sum(cls_sizes)
        extra = SEG_SLOTS // 2 if h == 0 else 0  # lead-in piece for class 0
        nseg_c = max(1, -(-(slots_c - extra) // SEG_SLOTS))
        target = (slots_c - extra) / nseg_c
        s_start, s_n = off, 0
        cap = extra if extra else target
        for bs in cls_sizes:
            if s_n and s_n + bs > min(cap + 64, SEG_SLOTS):
                segments.append((h, s_start, s_n))
                s_start += s_n
                s_n = 0
                cap = target
            s_n += bs
        segments.append((h, s_start, s_n))
        off += slots_c
    # taper: split the final segment so the serialized-transfer tail and the
    # last-consumer chain are short
    if segments[-1][2] > 3072:
        h, st, ns = segments.pop()
        goal = ns // 2
        acc, split, boff = 0, 0, 0
        for (hh, g, _j) in plan_batches:
            bs = 128 * g
            if st <= boff < st + ns:
                if acc + bs > goal and acc > 0:
                    split = boff
                    break
                acc += bs
            boff += bs
        if split:
            segments.append((h, st, split - st))
            segments.append((h, split, st + ns - split))
        else:
            segments.append((h, st, ns))

    # --- subtiles: chunks of 8 edge-planes (slots are plane-major per batch:
    # slot = batch_off + plane*128 + vnode, so each subtile is 128*n_planes
    # contiguous slots and the dense -P block repeats identically per plane)
    PLANES = SUB_SLOTS // 128  # 8
    subtiles = []
    off = 0
    last_sub_of_batch = []
    for bj, (h, g, _j) in enumerate(plan_batches):
        for i0 in range(0, g, PLANES):
            n_planes = min(PLANES, g - i0)
            subtiles.append(dict(batch=bj, i0=i0, n_planes=n_planes,
                                 slot=off + i0 * 128, g=g))
        off += 128 * g
        last_sub_of_batch.append(len(subtiles) - 1)
    # attach segment id to each subtile
    seg_of_slot = np.zeros(tot_slots + 1, dtype=np.int64)
    for si, (_h, st, ns) in enumerate(segments):
        seg_of_slot[st:st + ns] = si
    for t in subtiles:
        t["seg"] = int(seg_of_slot[t["slot"]])

    plan = dict(batches=plan_batches, segments=segments, subtiles=subtiles,
                nbatch=nbatch, tot_slots=tot_slots,
                last_sub_of_batch=last_sub_of_batch)

    # --- per-core packing
    per_core = []
    for c in range(NCORES):
        d = cores[c]
        nv = len(d["vdeg"])
        # sort this core's virtuals into plan order: class, then deg desc
        vorder = np.lexsort((-d["vdeg"], d["vcls"]))
        # per-class partition points in plan batches
        slot_blob = np.zeros(tot_slots, dtype=np.int16)
        vmap_node = np.full(nbatch * 128, -1, dtype=np.int64)  # virtual -> local node
        pad_per_node = np.zeros(LPC, dtype=np.int64)

        # iterate plan batches; batch (h, g, j) owns the j-th 128-chunk of
        # this core's degree-sorted class-h virtuals (order-independent)
        cls_sorted = {h: vorder[d["vcls"][vorder] == h] for h in (0, 1)}
        off = 0
        for bj, (h, g, j) in enumerate(plan_batches):
            lst = cls_sorted[h]
            take = lst[j * 128:(j + 1) * 128]
            for p, vi in enumerate(take):
                dg = int(d["vdeg"][vi])
                st = int(d["start"][vi])
                assert dg <= g
                # plane-major: vnode p's i-th edge at off + i*128 + p
                slot_blob[off + p:off + p + 128 * dg:128] = \
                    d["gg"][st:st + dg].astype(np.int16)
                # remaining g-dg planes stay 0 (dummy row of the class table)
                node = int(d["vnode"][vi])
                vmap_node[bj * 128 + p] = node
                pad_per_node[node] += g - dg
            off += 128 * g

        # wrapped idx layout for dma_gather: w[p, ccol] = blob[ccol*16 + p%16]
        wrapped = np.tile(slot_blob.reshape(-1, 16).T, (8, 1)).astype(np.int16)

        # per-virtual x (permuted, duplicated per virtual), feature-major +ones
        lpadv = nbatch * 128
        xpt = np.zeros((F_IN + 1, lpadv), dtype=np.float32)
        per_core.append(dict(wrapped=wrapped, xpt=xpt,
                             vmap_node=vmap_node, pad_per_node=pad_per_node,
                             deg=d["deg"], lpadv=lpadv))
    return plan, per_core


def _build_program(plan):
    nbatch = plan["nbatch"]
    tot = plan["tot_slots"]
    segs = plan["segments"]
    subs = plan["subtiles"]
    lpadv = nbatch * 128
    n_pchunk = (lpadv + 511) // 512
    max_g = max(g for (_h, g, _j) in plan["batches"])

    nc = bacc.Bacc("TRN2", num_swdge_queues=NQ)
    xpad_d = nc.dram_tensor("xpad", [TAB_ROWS, 2 * F_IN], BF16, kind="ExternalInput")
    xpt_d = nc.dram_tensor("xpt", [F_IN + 1, lpadv], BF16, kind="ExternalInput")
    aaug_d = nc.dram_tensor("aaug", [F_IN + 1, F_OUT], BF16, kind="ExternalInput")
    baug_d = nc.dram_tensor("baug", [2 * F_IN, F_OUT], BF16, kind="ExternalInput")
    idx_d = nc.dram_tensor("idx", [128, tot // 16], I16, kind="ExternalInput")
    sout_d = nc.dram_tensor("sout", [F_OUT, lpadv], F32, kind="ExternalOutput")
    npout_d = nc.dram_tensor("npout", [F_OUT, lpadv], BF16, kind="ExternalOutput")

    from contextlib import ExitStack

    with ExitStack() as ctx:
        block = ctx.enter_context(nc.Block())
        sb = lambda name, shape, dt: ctx.enter_context(nc.sbuf_tensor(name, shape, dt))
        ps = lambda name, shape: ctx.enter_context(nc.psum_tensor(name, shape, F32))
        sem = lambda name: ctx.enter_context(nc.semaphore(name))

        max_seg = max(ns for (_h, _st, ns) in segs)
        xg = [sb(f"xg{i}", [128, max_seg], BF16) for i in range(2 * NQ)]
        idxs = sb("idxs", [128, tot // 16], I16)
        np2 = sb("np2", [128, lpadv], BF16)            # -P per virtual (bf16)
        xpt_s = sb("xpt_s", [F_IN + 1, lpadv], BF16)
        qs0 = sb("qs0", [128, SUB_SLOTS], BF16)        # Q bf16 drain
        qs1 = sb("qs1", [128, SUB_SLOTS], BF16)
        m0 = sb("m0", [128, 128 * max_g], BF16)        # max() planes, per batch
        m1 = sb("m1", [128, 128 * max_g], BF16)
        rt0 = sb("rt0", [128, 128], F32)
        rt1 = sb("rt1", [128, 128], F32)
        aaug_s = sb("aaug_s", [F_IN + 1, F_OUT], BF16)
        baug_s = sb("baug_s", [2 * F_IN, F_OUT], BF16)
        pq0 = ps("pq0", [128, SUB_SLOTS])
        pq1 = ps("pq1", [128, SUB_SLOTS])
        pp0 = ps("pp0", [128, 512])
        pp1 = ps("pp1", [128, 512])
        s_in = sem("s_in")
        s_idx = sem("s_idx")
        s_gd = sem("s_gd")
        s_prep = sem("s_prep")
        s_mm = sem("s_mm")
        s_pp = sem("s_pp")
        s_npd = sem("s_npd")   # ACT drained -P chunk k into np2
        s_qd = sem("s_qd")
        s_tt = sem("s_tt")
        s_red = sem("s_red")
        s_out = [sem("s_out0"), sem("s_out1")]
        s_npo = sem("s_npo")
        qs = [qs0, qs1]
        m = [m0, m1]
        rt = [rt0, rt1]
        pq = [pq0, pq1]
        pp = [pp0, pp1]

        nseg = len(segs)
        nsub = len(subs)
        N_IN_DMAS = 3  # xpt, aaug, baug

        # last subtile index per segment (for gather buffer recycling)
        last_sub_of_seg = {}
        for t_i, t in enumerate(subs):
            last_sub_of_seg[t["seg"]] = t_i
        last_sub_of_batch = plan["last_sub_of_batch"]

        # greedy queue assignment balances desc-gen across the 4 pairs;
        # 2 xg buffers per queue so a trigger only waits on the segment
        # two-back on its queue (keeps desc-gen pairs from idling)
        qloads = [0] * NQ
        qa, xbuf = [], []
        _hist = [[] for _ in range(NQ)]
        prev2_seg = []
        for _si, (_h, _st, _ns) in enumerate(segs):
            _q = min(range(NQ), key=lambda i: qloads[i])
            qa.append(_q)
            qloads[_q] += _ns
            _k = len(_hist[_q])
            xbuf.append(2 * _q + _k % 2)
            prev2_seg.append(_hist[_q][-2] if _k >= 2 else None)
            _hist[_q].append(_si)

        @block.sync
        def _(sync):
            sync.dma_start(idxs[:, :], idx_d[:, :]).then_inc(s_idx, 16)
            sync.dma_start(aaug_s[:, :], aaug_d[:, :]).then_inc(s_in, 16)
            sync.dma_start(baug_s[:, :], baug_d[:, :]).then_inc(s_in, 16)
            sync.dma_start(xpt_s[:, :], xpt_d[:, :]).then_inc(s_in, 16)
            for j in range(nbatch):
                sync.wait_ge(s_red, j + 1)
                sync.dma_start(sout_d[:, 128 * j:128 * (j + 1)],
                               rt[j % 2][:, :]).then_inc(s_out[j % 2], 16)
            # -P to DRAM for the host-side P fold (exact bf16 match with TT)
            sync.dma_start(npout_d[:, :], np2[:, :]).then_inc(s_npo, 16)

        @block.gpsimd
        def _(gp):
            # Descriptor generation overlaps across the NQ Q7 queue pairs
            # (prepare_only). DMA execution: groups of GROUP segments fire
            # back-to-back on one queue (count=N trigger; same-queue rx
            # streams are fenced by each gather's xbar flush) while group
            # boundaries stay strictly serialized -- concurrent cross-queue
            # rx streams corrupt the shared xbar staging (measured).
            gp.load_library(mlp_lib)

            def prep(si):
                h, st, ns = segs[si]
                base = 0 if h == 0 else HI_BASE
                nrows = (HI_BASE if h == 0 else TAB_ROWS) - base
                gp.wait_ge(s_idx, 16)
                gp.dma_gather(
                    xg[xbuf[si]][:, :ns].rearrange("p (a s) -> p a s", a=1),
                    xpad_d[base:base + nrows, :],
                    idxs[:, st // 16:(st + ns) // 16],
                    ns, ns, 2 * F_IN,
                    transpose=True,
                    single_packet=False,
                    queue_num=qa[si],
                    prepare_only=True,
                    sem=s_gd,
                ).then_inc(s_prep, 1)

            def trig(si):
                gp.wait_ge(s_prep, si + 1)
                if si >= 1:
                    gp.wait_ge(s_gd, 16 * si)  # previous segment's DMA done
                if prev2_seg[si] is not None:
                    # xg[xbuf[si]] free only after PE consumed the segment
                    # two-back on this queue (double-buffered per queue)
                    gp.wait_ge(s_mm, last_sub_of_seg[prev2_seg[si]] + 1)
                gp.trigger_dma(count=1, queue_num=qa[si])

            # order: p0..p3 t0 t1 [p4 t2] [p5 t3] ... [p_j t_{j-2}] ... t_{n-2} t_{n-1}
            for si in range(min(NQ, nseg)):
                prep(si)
            trig(0)
            if nseg > 1:
                trig(1)
            for si in range(NQ, nseg):
                prep(si)
                trig(si - 2)
            for si in range(max(nseg - 2, 2), nseg):
                trig(si)

        @block.tensor
        def _(pe):
            pe.wait_ge(s_in, 16 * N_IN_DMAS)
            # P_T = Aaug.T @ xpt  (per-virtual P, feature-major)
            for k in range(n_pchunk):
                w = min(512, lpadv - 512 * k)
                if k >= 2:
                    pe.wait_ge(s_npd, k - 1)  # pp[k%2] free after ACT drained it
                pe.matmul(pp[k % 2][:, :w], aaug_s[:, :],
                          xpt_s[:, 512 * k:512 * k + w],
                          start=True, stop=True).then_inc(s_pp)
            # main loop: MLP matmuls
            for t_i, t in enumerate(subs):
                ncols = 128 * t["n_planes"]
                sg = t["seg"]
                pe.wait_ge(s_gd, 16 * (sg + 1))
                if t_i >= 2:
                    pe.wait_ge(s_qd, t_i - 1)  # pq[t_i%2] free after ACT drain
                soff = t["slot"] - segs[sg][1]
                # one matmul per PSUM bank (max 512 fp32 output columns)
                for c0 in range(0, ncols, 512):
                    w = min(512, ncols - c0)
                    mm = pe.matmul(pq[t_i % 2][:, c0:c0 + w], baug_s[:, :],
                                   xg[xbuf[sg]][:, soff + c0:soff + c0 + w],
                                   start=True, stop=True)
                    if c0 + w == ncols:
                        mm.then_inc(s_mm)

        @block.scalar
        def _(act):
            # -P drain: PSUM fp32 -> np2 bf16 (scale=-1 negates for free)
            for k in range(n_pchunk):
                w = min(512, lpadv - 512 * k)
                act.wait_ge(s_pp, k + 1)
                act.activation(np2[:, 512 * k:512 * k + w], pp[k % 2][:, :w],
                               mybir.ActivationFunctionType.Copy,
                               scale=-1.0).then_inc(s_npd)
            # Q drain: PSUM fp32 -> SBUF bf16
            for t_i, t in enumerate(subs):
                ncols = 128 * t["n_planes"]
                act.wait_ge(s_mm, t_i + 1)
                if t_i >= 2:
                    act.wait_ge(s_tt, t_i - 1)  # qs[t_i%2] free after DVE max
                act.activation(qs[t_i % 2][:, :ncols], pq[t_i % 2][:, :ncols],
                               mybir.ActivationFunctionType.Copy).then_inc(s_qd)

        @block.vector
        def _(dve):
            # dense max per subtile into the batch-wide plane buffer, then a
            # dense plane-fold tree per batch (tensor_tensor add stays in 2x
            # perf mode; tensor_reduce with strided planes would run ~1.6x
            # slower and a vnode-major layout would cost 1x on the max)
            def emit_reduce(bj):
                g = plan["batches"][bj][1]
                mb = m[bj % 2]
                dve.wait_ge(s_tt, last_sub_of_batch[bj] + 1)  # planes retired
                if bj >= 2:
                    dve.wait_ge(s_out[bj % 2], 16 * (bj // 2))  # rt[bj%2] free
                np_ = g
                while np_ > 1:
                    folded = np_ // 2        # planes folded down
                    keep = np_ - folded      # planes surviving (>= folded)
                    op = dve.tensor_tensor(
                        rt[bj % 2][:, :] if np_ == 2 else mb[:, :128 * folded],
                        mb[:, :128 * folded],
                        mb[:, 128 * keep:128 * np_],
                        op=mybir.AluOpType.add,
                    )
                    np_ = keep
                    if np_ == 1:
                        op.then_inc(s_red)
                if g == 1:
                    dve.tensor_scalar_mul(rt[bj % 2][:, :], mb[:, :128],
                                          1.0).then_inc(s_red)

            for t_i, t in enumerate(subs):
                ncols = 128 * t["n_planes"]
                bj = t["batch"]
                dve.wait_ge(s_qd, t_i + 1)
                if t["i0"] == 0:
                    k_need = ((bj + 1) * 128 - 1) // 512
                    dve.wait_ge(s_npd, k_need + 1)  # np2 block drained
                    if bj >= 2:
                        dve.wait_ge(s_red, bj - 1)  # m[bj%2] free after red(bj-2)
                dve.tensor_tensor(
                    m[bj % 2][:, 128 * t["i0"]:128 * t["i0"] + ncols],
                    qs[t_i % 2][:, :ncols],
                    np2[:, 128 * bj:128 * (bj + 1)]
                        .rearrange("p (one n) -> p one n", one=1)
                        .to_broadcast([128, t["n_planes"], 128]),
                    op=mybir.AluOpType.max,
                ).then_inc(s_tt)
                # reduce of the previous batch, one TT behind so the s_tt wait
                # inside emit_reduce is already satisfied (no pipeline stall)
                if t["i0"] == 0 and bj >= 1:
                    emit_reduce(bj - 1)
            emit_reduce(nbatch - 1)

    nc.compile()
    return nc


_CACHE = {}
TRACE = False
LAST_EXEC_NS = None


def kernel(x, edge_index, W, b):
    x = np.asarray(x, dtype=np.float32)
    W = np.asarray(W, dtype=np.float32)
    b = np.asarray(b, dtype=np.float32)
    plan, per_core = _plan_and_pack(edge_index)

    key = (plan["tot_slots"], plan["nbatch"], tuple(plan["batches"]))
    if key not in _CACHE:
        _CACHE[key] = _build_program(plan)
    nc = _CACHE[key]

    # ---- global tables
    W1, W2 = W[:, :F_IN], W[:, F_IN:]
    A = (W1 - W2).T.astype(np.float32)          # [64, 128]
    B = W2.T.astype(np.float32)                 # [64, 128]
    aaug = np.concatenate([A, b[None, :]], axis=0).astype(ml_dtypes.bfloat16)
    baug = np.zeros((2 * F_IN, F_OUT), dtype=np.float32)
    baug[:F_IN] = B
    baug[DUMMY_CH, :] = NEG_BIG
    baug = baug.astype(ml_dtypes.bfloat16)

    xpad = np.zeros((TAB_ROWS, 2 * F_IN), dtype=ml_dtypes.bfloat16)
    xb = x.astype(ml_dtypes.bfloat16)
    xpad[1:1 + CLASS_SPLIT, :F_IN] = xb[:CLASS_SPLIT]
    xpad[HI_BASE + 1:HI_BASE + 1 + (N_NODES - CLASS_SPLIT), :F_IN] = xb[CLASS_SPLIT:]
    xpad[0, DUMMY_CH] = 1.0
    xpad[HI_BASE, DUMMY_CH] = 1.0

    in_maps = []
    for c in range(NCORES):
        pc = per_core[c]
        # per-virtual x columns (feature-major, ones row for bias)
        vmap = pc["vmap_node"]
        xpt = pc["xpt"]
        valid = vmap >= 0
        gl = np.zeros(len(vmap), dtype=np.int64)
        gl[valid] = vmap[valid] + c * LPC
        xpt[:F_IN, :] = np.where(valid[None, :], x[gl].T, 0.0)
        xpt[F_IN, :] = np.where(valid, 1.0, 0.0)
        in_maps.append({
            "xpad": xpad, "xpt": xpt.astype(ml_dtypes.bfloat16),
            "aaug": aaug, "baug": baug,
            "idx": pc["wrapped"],
        })

    global LAST_EXEC_NS
    res = run_bass_kernel_spmd(nc, in_maps, core_ids=list(range(NCORES)),
                               trace=TRACE)
    if TRACE:
        LAST_EXEC_NS = res.exec_time_ns

    # ---- assembly
    out = np.zeros((N_NODES, F_OUT), dtype=np.float32)
    for c in range(NCORES):
        pc = per_core[c]
        RT = res.results[c]["sout"]         # [128, lpadv] = R (unscaled) per virtual
        NPT = res.results[c]["npout"]       # [128, lpadv] = -P per virtual (bf16)
        vmap = pc["vmap_node"]
        valid = vmap >= 0
        deg = pc["deg"]                     # true degree per local node
        pad = pc["pad_per_node"]
        acc = np.zeros((LPC, F_OUT), dtype=np.float32)
        np.add.at(acc, vmap[valid], RT.T[valid])
        # P per local node (first virtual of each node carries it); bf16 -P is
        # exactly what the device max() used, so pad-slot terms cancel exactly
        P_loc = np.zeros((LPC, F_OUT), dtype=np.float32)
        P_loc[vmap[valid]] = -NPT.T[valid].astype(np.float32)
        invdeg = (1.0 / np.maximum(deg, 1))[:, None].astype(np.float32)
        c1 = (1.0 + pad[:, None] * invdeg).astype(np.float32)
        loc = P_loc * c1 + acc * invdeg
        loc[deg == 0] = 0.0
        out[c * LPC:(c + 1) * LPC] = loc
    return out

